# revision 33
# baseline (speedup 1.0000x reference)
"""Trainium2 Bass kernel for nn_Embed_38766374814290 (embedding_lookup).

Math: out[i,j,l,e] = A[m][e] + delta_s[i,j,l] * B[m][e]
  where m = (j < traj_len[i]), delta_s = where(m, mat2[traj_loc-1], 0),
  A[m] = emb_sl_w[m] + emb_tl_w[m],
  B[m] = (emb_su_w[m]-emb_sl_w[m])/SU + (emb_tu_w[m]-emb_tl_w[m])/TU.

Sharding: pure data parallel over batch N = 32 -> 4 rows per core x 8 cores.

The rel-err gate is 2e-2; bf16 output rounding is ~2^-9, so the device
computes and stores the output in bf16 (halving HBM write traffic vs
fp32 -> ~23us DMA roofline per core) and the host upcasts to fp32.

Per-core pipeline, per batch row i, per pair c of 32-position blocks:
  1. One indirect pair-gather pulls mat2x rows for 64 positions into
     gsw[0:64, 128c:+128] (invalid positions redirect to an appended
     all-zero row 4096). The SWDGE descriptor path needs no gpsimd
     ucode library, so gathers start ~9.5us (DMAGatherAnt stalls ~12us
     on a lazy library load).
  2. A DVE stream-transpose (in-place 32x32 blocks) turns the window
     into lhsT layout: even j at partitions 0:32, odd j at 32:64. One
     host-prepared DMA fills rows 64:68 = [m_even, 1, m_odd, 1] and
     zero-pads rows 68:128.
  3. Matmul (i,j,s): lhsT = the full [128, 128] window (K=128: K<=64
     tiles run at half PE column throughput); one of 8 rhs variants
     selects the even/odd band and adds A[m] via the m/1 rows. Out
     partition f = 32g+b carries (pos=32j+b, l-group g). Two s-matmuls
     (F=512) per 2-bank PSUM tile, two tiles per group, so the
     Activation and Vector engines evict the halves in parallel and
     4-tile WAR depth keeps the PE fed.
  4. Output DMA per (i, j) writes the permuted [128, 2048] tile as-is
     (128 x 4KiB contiguous descriptors over all 16 DMA queues); the
     host gather step undoes the (g,b) permutation while upcasting.
"""
import os
import numpy as np
from contextlib import ExitStack

SU, TU = 10000.0, 86400.0
N, M, L, E = 32, 128, 128, 64
NLOC = 4096
NCORES = 8
ROWS = N // NCORES  # 4 batch rows per core

_CACHE = {}


def _install_profhook():
    """Optional: shim the missing antenv.axon_hooks so trace=True works."""
    import sys
    import types
    if "antenv.axon_hooks" in sys.modules:
        return True
    try:
        from trn_agent_boot.trn_boot import _ntff_profile_via_ctypes
    except Exception:
        return False
    hook = [None]
    mod = types.ModuleType("antenv.axon_hooks")
    mod.set_axon_ntff_profile_hook = lambda h: hook.__setitem__(0, h)
    mod.get_axon_ntff_profile_hook = lambda: hook[0]
    sys.modules["antenv.axon_hooks"] = mod
    try:
        mod.set_axon_ntff_profile_hook(
            _ntff_profile_via_ctypes("/opt/axon/libaxon_pjrt.so"))
    except Exception:
        return False
    return True


def _build():
    import concourse.bass as bass
    import concourse.tile as tile
    from concourse import bacc, mybir

    F32 = mybir.dt.float32
    BF16 = mybir.dt.bfloat16
    I32 = mybir.dt.int32

    nc = bacc.Bacc("TRN2", target_bir_lowering=False, debug=False,
                   enable_asserts=False, num_devices=NCORES)
    m2_d = nc.dram_tensor("m2", [NLOC + 1, L], BF16,
                          kind="ExternalInput").ap()
    offs_d = nc.dram_tensor("offs", [64, 2 * ROWS], I32,
                            kind="ExternalInput").ap()
    mrow_d = nc.dram_tensor("mrow", [96, ROWS * 256], BF16,
                            kind="ExternalInput").ap()
    rhs_d = nc.dram_tensor("rhs", [8, 128, 8 * E], BF16,
                           kind="ExternalInput").ap()
    # device-side layout keeps the matmul partition permutation:
    # out[i, j, 32g+b, 512s+64lp+e] = result(pos=32j+b, l=32g+8s+lp, e)
    out_d = nc.dram_tensor("out", [ROWS, 4, M, 4 * 8 * E], BF16,
                           kind="ExternalOutput").ap()

    with tile.TileContext(nc) as tc, ExitStack() as ctx:
        const = ctx.enter_context(tc.tile_pool(name="const", bufs=1))
        gpool = ctx.enter_context(tc.tile_pool(name="gath", bufs=2))
        opool = ctx.enter_context(tc.tile_pool(name="orow", bufs=6))
        pso = ctx.enter_context(tc.tile_pool(name="pso", bufs=4, space="PSUM"))

        offt = const.tile([64, 2 * ROWS], I32)
        nc.scalar.dma_start(offt[:], offs_d[:])
        # gtrow: one 128-col window per gather PAIR (i, c); partitions
        # 0:32 = even-j G^T, 32:64 = odd-j G^T, 64:68 = [m_e, 1, m_o, 1],
        # 68:128 zero. K=128 matmuls read the full window; two rhs
        # variants select the even or odd band (K<=64 tiles would run at
        # half column throughput, so always use the full array).
        gtrow = const.tile([128, ROWS * 256], BF16)
        nc.scalar.dma_start(gtrow[64:128, :], mrow_d[32:96, :])
        rhs_tiles = []
        for v in range(8):
            rt = const.tile([128, 8 * E], BF16, tag=f"rhs{v}")
            nc.scalar.dma_start(rt[:], rhs_d[v])
            rhs_tiles.append(rt)

        # HAM warmup: a few matmuls gated on the offs DMA (~9.5us) keep
        # the PE clock ramping until the first real matmul arrives.
        # Results are never read.
        wrhs = const.tile([64, 8 * E], BF16)
        nc.vector.memset(wrhs[:], 0.0)
        wpo = pso.tile([128, 2 * 8 * E], F32, tag="po")
        wlhs = offt[:].bitcast(BF16)
        for _ in range(9):
            nc.tensor.matmul(wpo[0:16, 0:512], lhsT=wlhs[:, 0:16],
                             rhs=wrhs[:], start=True, stop=True)

        # evict pattern per group: (ACT, DVE) halves in parallel; two
        # groups go (ACT, ACT) to offload the stream-transpose-loaded DVE
        act_both = {6}

        for i in range(ROWS):
            gsw = gpool.tile([64, 256], BF16)
            for c in range(2):
                # each pair-gather pulls 64 positions (two j-blocks on
                # partitions 0:32 / 32:64); the transpose interleaves
                # with eviction casts on the in-order DVE queue
                nc.gpsimd.indirect_dma_start(
                    out=gsw[:, 128 * c:128 * (c + 1)], out_offset=None,
                    in_=m2_d[:],
                    in_offset=bass.IndirectOffsetOnAxis(
                        ap=offt[:, 2 * i + c:2 * i + c + 1], axis=0))
                nc.vector.transpose(
                    out=gtrow[0:64, 256 * i + 128 * c:256 * i + 128 * (c + 1)],
                    in_=gsw[:, 128 * c:128 * (c + 1)])
            for j in range(4):
                c, pb = j >> 1, j & 1
                w = 256 * i + 128 * c
                k = 4 * i + j
                orow = opool.tile([128, 4 * 8 * E], BF16)
                for half in range(2):
                    po = pso.tile([128, 2 * 8 * E], F32, tag="po")
                    for sp in range(2):
                        s = 2 * half + sp
                        nc.tensor.matmul(po[:, 512 * sp:512 * (sp + 1)],
                                         lhsT=gtrow[:, w:w + 128],
                                         rhs=rhs_tiles[4 * pb + s][:],
                                         start=True, stop=True)
                    dst = orow[:, 1024 * half:1024 * (half + 1)]
                    if half == 0 or k in act_both:
                        nc.scalar.copy(out=dst, in_=po[:])
                    else:
                        nc.vector.tensor_copy(out=dst, in_=po[:])
                if k < 2:
                    # the DMA stream end = stream start + queue work; the
                    # first groups ship per-half so the stream starts as
                    # soon as the first (ACT) eviction half lands
                    nc.sync.dma_start(out_d[i, j][:, 0:1024],
                                      orow[:, 0:1024])
                    nc.sync.dma_start(out_d[i, j][:, 1024:2048],
                                      orow[:, 1024:2048])
                else:
                    nc.sync.dma_start(out_d[i, j], orow[:])
    nc.compile()
    return nc


def kernel(traj_loc, mat2, vec, traj_len, l_max, emb_sl_w, emb_su_w,
           emb_tl_w, emb_tu_w):
    import ml_dtypes
    from concourse import bass_utils

    BF = ml_dtypes.bfloat16
    traj_loc = np.asarray(traj_loc).astype(np.int64)
    mat2 = np.ascontiguousarray(np.asarray(mat2, dtype=np.float32))
    traj_len = np.asarray(traj_len).astype(np.int64)
    esl = np.asarray(emb_sl_w, dtype=np.float32)
    esu = np.asarray(emb_su_w, dtype=np.float32)
    etl = np.asarray(emb_tl_w, dtype=np.float32)
    etu = np.asarray(emb_tu_w, dtype=np.float32)

    # host prep: constants
    A = esl + etl                                            # [2, E]
    B = (esu - esl) / np.float32(SU) + (etu - etl) / np.float32(TU)
    mask = (np.arange(M)[None, :] < traj_len[:, None])       # [N, M]
    idx_full = np.where(mask, traj_loc - 1, NLOC).astype(np.int32)

    b1 = B[1].astype(BF)
    dA = (A[1] - A[0]).astype(BF)
    a0 = A[0].astype(BF)

    mat2x = np.concatenate([mat2, np.zeros((1, L), np.float32)], axis=0)
    m2 = np.ascontiguousarray(mat2x.astype(BF))

    # gather offsets: pair-gather (i, c) row-gathers mat2x[idx[i, 64c+p]]
    # into partition p (0:64) of gsw[:, 128c:128c+128]
    offs = np.empty((NCORES, 64, 2 * ROWS), np.int32)
    for cc in range(NCORES):
        for i in range(ROWS):
            idx = idx_full[ROWS * cc + i]                    # [128]
            for c in range(2):
                offs[cc, :, 2 * i + c] = idx[64 * c:64 * (c + 1)]

    # rhs[4*pb+s]: variant pb selects the even (rows 0:32) or odd (rows
    # 32:64) G^T band: row 32*pb+8s+lp has B1 in e-block lp; the [m, 1]
    # selectors at rows 64+2*pb : 66+2*pb add m*dA + A0 per e-block.
    rhs = np.zeros((8, 128, 8 * E), BF)
    for pb in range(2):
        for s in range(4):
            v = 4 * pb + s
            for lp in range(8):
                rhs[v, 32 * pb + 8 * s + lp, E * lp:E * (lp + 1)] = b1
            rhs[v, 64 + 2 * pb, :] = np.tile(dA, 8)
            rhs[v, 65 + 2 * pb, :] = np.tile(a0, 8)

    # gtrow rows 64:68 per (i, c) window: [m_even, 1, m_odd, 1] where
    # m_j col 32g+b = mask[pos=32j+b] tiled over g; the rest of the
    # staging buffer zero-pads gtrow rows 68:128 for K=128 matmuls.
    # mrow staging covers gtrow rows 32:128 -> buffer rows 32:36 hold it.
    mrow_full = np.zeros((NCORES, 96, ROWS * 256), BF)
    for cc in range(NCORES):
        mc = mask[ROWS * cc:ROWS * (cc + 1)].astype(BF)      # [ROWS, 128]
        mj = np.tile(mc.reshape(ROWS, 4, 1, 32), (1, 1, 4, 1))  # [R,4j,4g,32]
        mjf = mj.reshape(ROWS, 4, 128)                       # [R, j, 128]
        for i in range(ROWS):
            for c in range(2):
                w = 256 * i + 128 * c
                mrow_full[cc, 32, w:w + 128] = mjf[i, 2 * c]
                mrow_full[cc, 33, w:w + 128] = 1.0
                mrow_full[cc, 34, w:w + 128] = mjf[i, 2 * c + 1]
                mrow_full[cc, 35, w:w + 128] = 1.0

    if "nc" not in _CACHE:
        _CACHE["nc"] = _build()
    nc = _CACHE["nc"]

    in_maps = []
    for c in range(NCORES):
        in_maps.append({
            "m2": m2,
            "offs": np.ascontiguousarray(offs[c]),
            "mrow": np.ascontiguousarray(mrow_full[c]),
            "rhs": rhs,
        })

    trace = os.environ.get("KERNEL_TRACE", "0") == "1" and _install_profhook()
    res = bass_utils.run_bass_kernel_spmd(
        nc, in_maps, core_ids=list(range(NCORES)), trace=bool(trace))
    if trace:
        _CACHE["exec_time_ns"] = res.exec_time_ns
        _CACHE["trace_path"] = (res.instructions_and_trace or (None, None))[1]
        _CACHE["tmpdir"] = res.profile_json

    # undo the device layout: [i, j, g, b, s, lp, e] -> [i, (j b), (g s lp), e]
    out = np.concatenate(
        [res.results[c]["out"].reshape(ROWS, 4, 4, 32, 4, 8, E)
         .transpose(0, 1, 3, 2, 4, 5, 6).reshape(ROWS, M, L, E)
         for c in range(NCORES)],
        axis=0).astype(np.float32)
    return out


# revision 34
# speedup vs baseline: 1.1318x; 1.1318x over previous
"""Trainium2 Bass kernel for nn_Embed_38766374814290 (embedding_lookup).

Math: out[i,j,l,e] = A[m][e] + delta_s[i,j,l] * B[m][e]
  where m = (j < traj_len[i]), delta_s = where(m, mat2[traj_loc-1], 0),
  A[m] = emb_sl_w[m] + emb_tl_w[m],
  B[m] = (emb_su_w[m]-emb_sl_w[m])/SU + (emb_tu_w[m]-emb_tl_w[m])/TU.

Sharding: pure data parallel over batch N = 32 -> 4 rows per core x 8 cores.

The rel-err gate is 2e-2; bf16 output rounding is ~2^-9, so the device
computes and stores the output in bf16 (halving HBM write traffic vs
fp32 -> ~23us DMA roofline per core) and the host upcasts to fp32.

Per-core pipeline, per batch row i, per pair c of 32-position blocks:
  1. One indirect pair-gather pulls mat2x rows for 64 positions into
     gsw[0:64, 128c:+128] (invalid positions redirect to an appended
     all-zero row 4096). The SWDGE descriptor path needs no gpsimd
     ucode library, so gathers start ~9.5us (DMAGatherAnt stalls ~12us
     on a lazy library load).
  2. A DVE stream-transpose (in-place 32x32 blocks) turns the window
     into lhsT layout: even j at partitions 0:32, odd j at 32:64. One
     host-prepared DMA fills rows 64:68 = [m_even, 1, m_odd, 1] and
     zero-pads rows 68:128.
  3. Matmul (i,j,s): lhsT = the full [128, 128] window (K=128: K<=64
     tiles run at half PE column throughput); one of 8 rhs variants
     selects the even/odd band and adds A[m] via the m/1 rows. Out
     partition f = 32g+b carries (pos=32j+b, l-group g). Two s-matmuls
     (F=512) per 2-bank PSUM tile, two tiles per group, so the
     Activation and Vector engines evict the halves in parallel and
     4-tile WAR depth keeps the PE fed.
  4. Output DMA per (i, j) writes the permuted [128, 2048] tile as-is
     (128 x 4KiB contiguous descriptors over all 16 DMA queues); the
     host gather step undoes the (g,b) permutation while upcasting.
"""
import os
import numpy as np
from contextlib import ExitStack

SU, TU = 10000.0, 86400.0
N, M, L, E = 32, 128, 128, 64
NLOC = 4096
NCORES = 8
ROWS = N // NCORES  # 4 batch rows per core

_CACHE = {}


def _install_profhook():
    """Optional: shim the missing antenv.axon_hooks so trace=True works."""
    import sys
    import types
    if "antenv.axon_hooks" in sys.modules:
        return True
    try:
        from trn_agent_boot.trn_boot import _ntff_profile_via_ctypes
    except Exception:
        return False
    hook = [None]
    mod = types.ModuleType("antenv.axon_hooks")
    mod.set_axon_ntff_profile_hook = lambda h: hook.__setitem__(0, h)
    mod.get_axon_ntff_profile_hook = lambda: hook[0]
    sys.modules["antenv.axon_hooks"] = mod
    try:
        mod.set_axon_ntff_profile_hook(
            _ntff_profile_via_ctypes("/opt/axon/libaxon_pjrt.so"))
    except Exception:
        return False
    return True


def _build():
    import concourse.bass as bass
    import concourse.tile as tile
    from concourse import bacc, mybir

    F32 = mybir.dt.float32
    BF16 = mybir.dt.bfloat16
    I32 = mybir.dt.int32

    nc = bacc.Bacc("TRN2", target_bir_lowering=False, debug=False,
                   enable_asserts=False, num_devices=NCORES)
    m2_d = nc.dram_tensor("m2", [NLOC + 1, L], BF16,
                          kind="ExternalInput").ap()
    offs_d = nc.dram_tensor("offs", [64, 2 * ROWS], I32,
                            kind="ExternalInput").ap()
    mrow_d = nc.dram_tensor("mrow", [96, ROWS * 256], BF16,
                            kind="ExternalInput").ap()
    rhs_d = nc.dram_tensor("rhs", [8, 128, 8 * E], BF16,
                           kind="ExternalInput").ap()
    # device-side layout keeps the matmul partition permutation:
    # out[i, j, 32g+b, 512s+64lp+e] = result(pos=32j+b, l=32g+8s+lp, e)
    out_d = nc.dram_tensor("out", [ROWS, 4, M, 4 * 8 * E], BF16,
                           kind="ExternalOutput").ap()

    with tile.TileContext(nc) as tc, ExitStack() as ctx:
        const = ctx.enter_context(tc.tile_pool(name="const", bufs=1))
        gpool = ctx.enter_context(tc.tile_pool(name="gath", bufs=2))
        opool = ctx.enter_context(tc.tile_pool(name="orow", bufs=6))
        pso = ctx.enter_context(tc.tile_pool(name="pso", bufs=4, space="PSUM"))

        offt = const.tile([64, 2 * ROWS], I32)
        nc.scalar.dma_start(offt[:], offs_d[:])
        # gtrow: one 128-col window per gather PAIR (i, c); partitions
        # 0:32 = even-j G^T, 32:64 = odd-j G^T, 64:68 = [m_e, 1, m_o, 1],
        # 68:128 zero. K=128 matmuls read the full window; two rhs
        # variants select the even or odd band (K<=64 tiles would run at
        # half column throughput, so always use the full array).
        gtrow = const.tile([128, ROWS * 256], BF16)
        nc.scalar.dma_start(gtrow[64:128, :], mrow_d[32:96, :])
        rhs_tiles = []
        for v in range(8):
            rt = const.tile([128, 8 * E], BF16, tag=f"rhs{v}")
            nc.scalar.dma_start(rt[:], rhs_d[v])
            rhs_tiles.append(rt)

        # HAM warmup: a few matmuls gated on the offs DMA (~9.5us) keep
        # the PE clock ramping until the first real matmul arrives.
        # Results are never read.
        wrhs = const.tile([64, 8 * E], BF16)
        nc.vector.memset(wrhs[:], 0.0)
        wpo = pso.tile([128, 2 * 8 * E], F32, tag="po")
        wlhs = offt[:].bitcast(BF16)
        for _ in range(9):
            nc.tensor.matmul(wpo[0:16, 0:512], lhsT=wlhs[:, 0:16],
                             rhs=wrhs[:], start=True, stop=True)

        # evict pattern per group: (ACT, DVE) halves in parallel; two
        # groups go (ACT, ACT) to offload the stream-transpose-loaded DVE
        act_both = {6}

        for i in range(ROWS):
            gsw = gpool.tile([64, 256], BF16)
            for c in range(2):
                # each pair-gather pulls 64 positions (two j-blocks on
                # partitions 0:32 / 32:64); the transpose interleaves
                # with eviction casts on the in-order DVE queue
                nc.gpsimd.indirect_dma_start(
                    out=gsw[:, 128 * c:128 * (c + 1)], out_offset=None,
                    in_=m2_d[:],
                    in_offset=bass.IndirectOffsetOnAxis(
                        ap=offt[:, 2 * i + c:2 * i + c + 1], axis=0))
                nc.vector.transpose(
                    out=gtrow[0:64, 256 * i + 128 * c:256 * i + 128 * (c + 1)],
                    in_=gsw[:, 128 * c:128 * (c + 1)])
            for j in range(4):
                c, pb = j >> 1, j & 1
                w = 256 * i + 128 * c
                k = 4 * i + j
                orow = opool.tile([128, 4 * 8 * E], BF16)
                for half in range(2):
                    po = pso.tile([128, 2 * 8 * E], F32, tag="po")
                    for sp in range(2):
                        s = 2 * half + sp
                        nc.tensor.matmul(po[:, 512 * sp:512 * (sp + 1)],
                                         lhsT=gtrow[:, w:w + 128],
                                         rhs=rhs_tiles[4 * pb + s][:],
                                         start=True, stop=True)
                    dst = orow[:, 1024 * half:1024 * (half + 1)]
                    if half == 0 or k in act_both:
                        nc.scalar.copy(out=dst, in_=po[:])
                    else:
                        nc.vector.tensor_copy(out=dst, in_=po[:])
                if k < 2 or k >= 14:
                    # the DMA stream end = stream start + queue work; the
                    # first groups ship per-half so the stream starts as
                    # soon as the first (ACT) eviction half lands, and
                    # the last groups ship per-half to shorten the tail
                    # before the teardown drain
                    nc.sync.dma_start(out_d[i, j][:, 0:1024],
                                      orow[:, 0:1024])
                    nc.sync.dma_start(out_d[i, j][:, 1024:2048],
                                      orow[:, 1024:2048])
                else:
                    nc.sync.dma_start(out_d[i, j], orow[:])
    nc.compile()
    return nc


def kernel(traj_loc, mat2, vec, traj_len, l_max, emb_sl_w, emb_su_w,
           emb_tl_w, emb_tu_w):
    import ml_dtypes
    from concourse import bass_utils

    BF = ml_dtypes.bfloat16
    traj_loc = np.asarray(traj_loc).astype(np.int64)
    mat2 = np.ascontiguousarray(np.asarray(mat2, dtype=np.float32))
    traj_len = np.asarray(traj_len).astype(np.int64)
    esl = np.asarray(emb_sl_w, dtype=np.float32)
    esu = np.asarray(emb_su_w, dtype=np.float32)
    etl = np.asarray(emb_tl_w, dtype=np.float32)
    etu = np.asarray(emb_tu_w, dtype=np.float32)

    # host prep: constants
    A = esl + etl                                            # [2, E]
    B = (esu - esl) / np.float32(SU) + (etu - etl) / np.float32(TU)
    mask = (np.arange(M)[None, :] < traj_len[:, None])       # [N, M]
    idx_full = np.where(mask, traj_loc - 1, NLOC).astype(np.int32)

    b1 = B[1].astype(BF)
    dA = (A[1] - A[0]).astype(BF)
    a0 = A[0].astype(BF)

    mat2x = np.concatenate([mat2, np.zeros((1, L), np.float32)], axis=0)
    m2 = np.ascontiguousarray(mat2x.astype(BF))

    # gather offsets: pair-gather (i, c) row-gathers mat2x[idx[i, 64c+p]]
    # into partition p (0:64) of gsw[:, 128c:128c+128]
    offs = np.empty((NCORES, 64, 2 * ROWS), np.int32)
    for cc in range(NCORES):
        for i in range(ROWS):
            idx = idx_full[ROWS * cc + i]                    # [128]
            for c in range(2):
                offs[cc, :, 2 * i + c] = idx[64 * c:64 * (c + 1)]

    # rhs[4*pb+s]: variant pb selects the even (rows 0:32) or odd (rows
    # 32:64) G^T band: row 32*pb+8s+lp has B1 in e-block lp; the [m, 1]
    # selectors at rows 64+2*pb : 66+2*pb add m*dA + A0 per e-block.
    rhs = np.zeros((8, 128, 8 * E), BF)
    for pb in range(2):
        for s in range(4):
            v = 4 * pb + s
            for lp in range(8):
                rhs[v, 32 * pb + 8 * s + lp, E * lp:E * (lp + 1)] = b1
            rhs[v, 64 + 2 * pb, :] = np.tile(dA, 8)
            rhs[v, 65 + 2 * pb, :] = np.tile(a0, 8)

    # gtrow rows 64:68 per (i, c) window: [m_even, 1, m_odd, 1] where
    # m_j col 32g+b = mask[pos=32j+b] tiled over g; the rest of the
    # staging buffer zero-pads gtrow rows 68:128 for K=128 matmuls.
    # mrow staging covers gtrow rows 32:128 -> buffer rows 32:36 hold it.
    mrow_full = np.zeros((NCORES, 96, ROWS * 256), BF)
    for cc in range(NCORES):
        mc = mask[ROWS * cc:ROWS * (cc + 1)].astype(BF)      # [ROWS, 128]
        mj = np.tile(mc.reshape(ROWS, 4, 1, 32), (1, 1, 4, 1))  # [R,4j,4g,32]
        mjf = mj.reshape(ROWS, 4, 128)                       # [R, j, 128]
        for i in range(ROWS):
            for c in range(2):
                w = 256 * i + 128 * c
                mrow_full[cc, 32, w:w + 128] = mjf[i, 2 * c]
                mrow_full[cc, 33, w:w + 128] = 1.0
                mrow_full[cc, 34, w:w + 128] = mjf[i, 2 * c + 1]
                mrow_full[cc, 35, w:w + 128] = 1.0

    if "nc" not in _CACHE:
        _CACHE["nc"] = _build()
    nc = _CACHE["nc"]

    in_maps = []
    for c in range(NCORES):
        in_maps.append({
            "m2": m2,
            "offs": np.ascontiguousarray(offs[c]),
            "mrow": np.ascontiguousarray(mrow_full[c]),
            "rhs": rhs,
        })

    trace = os.environ.get("KERNEL_TRACE", "0") == "1" and _install_profhook()
    res = bass_utils.run_bass_kernel_spmd(
        nc, in_maps, core_ids=list(range(NCORES)), trace=bool(trace))
    if trace:
        _CACHE["exec_time_ns"] = res.exec_time_ns
        _CACHE["trace_path"] = (res.instructions_and_trace or (None, None))[1]
        _CACHE["tmpdir"] = res.profile_json

    # undo the device layout: [i, j, g, b, s, lp, e] -> [i, (j b), (g s lp), e]
    out = np.concatenate(
        [res.results[c]["out"].reshape(ROWS, 4, 4, 32, 4, 8, E)
         .transpose(0, 1, 3, 2, 4, 5, 6).reshape(ROWS, M, L, E)
         for c in range(NCORES)],
        axis=0).astype(np.float32)
    return out


# revision 35
# speedup vs baseline: 1.1450x; 1.0116x over previous
"""Trainium2 Bass kernel for nn_Embed_38766374814290 (embedding_lookup).

Math: out[i,j,l,e] = A[m][e] + delta_s[i,j,l] * B[m][e]
  where m = (j < traj_len[i]), delta_s = where(m, mat2[traj_loc-1], 0),
  A[m] = emb_sl_w[m] + emb_tl_w[m],
  B[m] = (emb_su_w[m]-emb_sl_w[m])/SU + (emb_tu_w[m]-emb_tl_w[m])/TU.

Sharding: pure data parallel over batch N = 32 -> 4 rows per core x 8 cores.

The rel-err gate is 2e-2; bf16 output rounding is ~2^-9, so the device
computes and stores the output in bf16 (halving HBM write traffic vs
fp32 -> ~23us DMA roofline per core) and the host upcasts to fp32.

Per-core pipeline, per batch row i, per pair c of 32-position blocks:
  1. One indirect pair-gather pulls mat2x rows for 64 positions into
     gsw[0:64, 128c:+128] (invalid positions redirect to an appended
     all-zero row 4096). The SWDGE descriptor path needs no gpsimd
     ucode library, so gathers start ~9.5us (DMAGatherAnt stalls ~12us
     on a lazy library load).
  2. A DVE stream-transpose (in-place 32x32 blocks) turns the window
     into lhsT layout: even j at partitions 0:32, odd j at 32:64. One
     host-prepared DMA fills rows 64:68 = [m_even, 1, m_odd, 1] and
     zero-pads rows 68:128.
  3. Matmul (i,j,s): lhsT = the full [128, 128] window (K=128: K<=64
     tiles run at half PE column throughput); one of 8 rhs variants
     selects the even/odd band and adds A[m] via the m/1 rows. Out
     partition f = 32g+b carries (pos=32j+b, l-group g). Two s-matmuls
     (F=512) per 2-bank PSUM tile, two tiles per group, so the
     Activation and Vector engines evict the halves in parallel and
     4-tile WAR depth keeps the PE fed.
  4. Output DMA per (i, j) writes the permuted [128, 2048] tile as-is
     (128 x 4KiB contiguous descriptors over all 16 DMA queues); the
     host gather step undoes the (g,b) permutation while upcasting.
"""
import os
import numpy as np
from contextlib import ExitStack

SU, TU = 10000.0, 86400.0
N, M, L, E = 32, 128, 128, 64
NLOC = 4096
NCORES = 8
ROWS = N // NCORES  # 4 batch rows per core

_CACHE = {}


def _install_profhook():
    """Optional: shim the missing antenv.axon_hooks so trace=True works."""
    import sys
    import types
    if "antenv.axon_hooks" in sys.modules:
        return True
    try:
        from trn_agent_boot.trn_boot import _ntff_profile_via_ctypes
    except Exception:
        return False
    hook = [None]
    mod = types.ModuleType("antenv.axon_hooks")
    mod.set_axon_ntff_profile_hook = lambda h: hook.__setitem__(0, h)
    mod.get_axon_ntff_profile_hook = lambda: hook[0]
    sys.modules["antenv.axon_hooks"] = mod
    try:
        mod.set_axon_ntff_profile_hook(
            _ntff_profile_via_ctypes("/opt/axon/libaxon_pjrt.so"))
    except Exception:
        return False
    return True


def _build():
    import concourse.bass as bass
    import concourse.tile as tile
    from concourse import bacc, mybir

    F32 = mybir.dt.float32
    BF16 = mybir.dt.bfloat16
    I32 = mybir.dt.int32

    nc = bacc.Bacc("TRN2", target_bir_lowering=False, debug=False,
                   enable_asserts=False, num_devices=NCORES)
    m2_d = nc.dram_tensor("m2", [NLOC + 1, L], BF16,
                          kind="ExternalInput").ap()
    offs_d = nc.dram_tensor("offs", [64, 2 * ROWS], I32,
                            kind="ExternalInput").ap()
    mrow_d = nc.dram_tensor("mrow", [96, ROWS * 256], BF16,
                            kind="ExternalInput").ap()
    rhs_d = nc.dram_tensor("rhs", [8, 128, 8 * E], BF16,
                           kind="ExternalInput").ap()
    # device-side layout keeps the matmul partition permutation:
    # out[i, j, 32g+b, 512s+64lp+e] = result(pos=32j+b, l=32g+8s+lp, e)
    out_d = nc.dram_tensor("out", [ROWS, 4, M, 4 * 8 * E], BF16,
                           kind="ExternalOutput").ap()

    with tile.TileContext(nc) as tc, ExitStack() as ctx:
        const = ctx.enter_context(tc.tile_pool(name="const", bufs=1))
        gpool = ctx.enter_context(tc.tile_pool(name="gath", bufs=3))
        opool = ctx.enter_context(tc.tile_pool(name="orow", bufs=8))
        pso = ctx.enter_context(tc.tile_pool(name="pso", bufs=4, space="PSUM"))

        offt = const.tile([64, 2 * ROWS], I32)
        nc.scalar.dma_start(offt[:], offs_d[:])
        # gtrow: one 128-col window per gather PAIR (i, c); partitions
        # 0:32 = even-j G^T, 32:64 = odd-j G^T, 64:68 = [m_e, 1, m_o, 1],
        # 68:128 zero. K=128 matmuls read the full window; two rhs
        # variants select the even or odd band (K<=64 tiles would run at
        # half column throughput, so always use the full array).
        gtrow = const.tile([128, ROWS * 256], BF16)
        nc.scalar.dma_start(gtrow[64:128, :], mrow_d[32:96, :])
        rhs_tiles = []
        for v in range(8):
            rt = const.tile([128, 8 * E], BF16, tag=f"rhs{v}")
            nc.scalar.dma_start(rt[:], rhs_d[v])
            rhs_tiles.append(rt)

        # HAM warmup: a few matmuls gated on the offs DMA (~9.5us) keep
        # the PE clock ramping until the first real matmul arrives.
        # Results are never read.
        wrhs = const.tile([64, 8 * E], BF16)
        nc.vector.memset(wrhs[:], 0.0)
        wpo = pso.tile([128, 2 * 8 * E], F32, tag="po")
        wlhs = offt[:].bitcast(BF16)
        for _ in range(9):
            nc.tensor.matmul(wpo[0:16, 0:512], lhsT=wlhs[:, 0:16],
                             rhs=wrhs[:], start=True, stop=True)

        # evict pattern per group: (ACT, DVE) halves in parallel; two
        # groups go (ACT, ACT) to offload the stream-transpose-loaded DVE
        act_both = {6}

        for i in range(ROWS):
            gsw = gpool.tile([64, 256], BF16)
            for c in range(2):
                # each pair-gather pulls 64 positions (two j-blocks on
                # partitions 0:32 / 32:64); the transpose interleaves
                # with eviction casts on the in-order DVE queue
                nc.gpsimd.indirect_dma_start(
                    out=gsw[:, 128 * c:128 * (c + 1)], out_offset=None,
                    in_=m2_d[:],
                    in_offset=bass.IndirectOffsetOnAxis(
                        ap=offt[:, 2 * i + c:2 * i + c + 1], axis=0))
                nc.vector.transpose(
                    out=gtrow[0:64, 256 * i + 128 * c:256 * i + 128 * (c + 1)],
                    in_=gsw[:, 128 * c:128 * (c + 1)])
            for j in range(4):
                c, pb = j >> 1, j & 1
                w = 256 * i + 128 * c
                k = 4 * i + j
                orow = opool.tile([128, 4 * 8 * E], BF16)
                for half in range(2):
                    po = pso.tile([128, 2 * 8 * E], F32, tag="po")
                    for sp in range(2):
                        s = 2 * half + sp
                        nc.tensor.matmul(po[:, 512 * sp:512 * (sp + 1)],
                                         lhsT=gtrow[:, w:w + 128],
                                         rhs=rhs_tiles[4 * pb + s][:],
                                         start=True, stop=True)
                    dst = orow[:, 1024 * half:1024 * (half + 1)]
                    if half == 0 or k in act_both:
                        nc.scalar.copy(out=dst, in_=po[:])
                    else:
                        nc.vector.tensor_copy(out=dst, in_=po[:])
                if k < 2 or k >= 14:
                    # the DMA stream end = stream start + queue work; the
                    # first groups ship per-half so the stream starts as
                    # soon as the first (ACT) eviction half lands, and
                    # the last groups ship per-half to shorten the tail
                    # before the teardown drain
                    nc.sync.dma_start(out_d[i, j][:, 0:1024],
                                      orow[:, 0:1024])
                    nc.sync.dma_start(out_d[i, j][:, 1024:2048],
                                      orow[:, 1024:2048])
                else:
                    nc.sync.dma_start(out_d[i, j], orow[:])
    nc.compile()
    return nc


def kernel(traj_loc, mat2, vec, traj_len, l_max, emb_sl_w, emb_su_w,
           emb_tl_w, emb_tu_w):
    import ml_dtypes
    from concourse import bass_utils

    BF = ml_dtypes.bfloat16
    traj_loc = np.asarray(traj_loc).astype(np.int64)
    mat2 = np.ascontiguousarray(np.asarray(mat2, dtype=np.float32))
    traj_len = np.asarray(traj_len).astype(np.int64)
    esl = np.asarray(emb_sl_w, dtype=np.float32)
    esu = np.asarray(emb_su_w, dtype=np.float32)
    etl = np.asarray(emb_tl_w, dtype=np.float32)
    etu = np.asarray(emb_tu_w, dtype=np.float32)

    # host prep: constants
    A = esl + etl                                            # [2, E]
    B = (esu - esl) / np.float32(SU) + (etu - etl) / np.float32(TU)
    mask = (np.arange(M)[None, :] < traj_len[:, None])       # [N, M]
    idx_full = np.where(mask, traj_loc - 1, NLOC).astype(np.int32)

    b1 = B[1].astype(BF)
    dA = (A[1] - A[0]).astype(BF)
    a0 = A[0].astype(BF)

    mat2x = np.concatenate([mat2, np.zeros((1, L), np.float32)], axis=0)
    m2 = np.ascontiguousarray(mat2x.astype(BF))

    # gather offsets: pair-gather (i, c) row-gathers mat2x[idx[i, 64c+p]]
    # into partition p (0:64) of gsw[:, 128c:128c+128]
    offs = np.empty((NCORES, 64, 2 * ROWS), np.int32)
    for cc in range(NCORES):
        for i in range(ROWS):
            idx = idx_full[ROWS * cc + i]                    # [128]
            for c in range(2):
                offs[cc, :, 2 * i + c] = idx[64 * c:64 * (c + 1)]

    # rhs[4*pb+s]: variant pb selects the even (rows 0:32) or odd (rows
    # 32:64) G^T band: row 32*pb+8s+lp has B1 in e-block lp; the [m, 1]
    # selectors at rows 64+2*pb : 66+2*pb add m*dA + A0 per e-block.
    rhs = np.zeros((8, 128, 8 * E), BF)
    for pb in range(2):
        for s in range(4):
            v = 4 * pb + s
            for lp in range(8):
                rhs[v, 32 * pb + 8 * s + lp, E * lp:E * (lp + 1)] = b1
            rhs[v, 64 + 2 * pb, :] = np.tile(dA, 8)
            rhs[v, 65 + 2 * pb, :] = np.tile(a0, 8)

    # gtrow rows 64:68 per (i, c) window: [m_even, 1, m_odd, 1] where
    # m_j col 32g+b = mask[pos=32j+b] tiled over g; the rest of the
    # staging buffer zero-pads gtrow rows 68:128 for K=128 matmuls.
    # mrow staging covers gtrow rows 32:128 -> buffer rows 32:36 hold it.
    mrow_full = np.zeros((NCORES, 96, ROWS * 256), BF)
    for cc in range(NCORES):
        mc = mask[ROWS * cc:ROWS * (cc + 1)].astype(BF)      # [ROWS, 128]
        mj = np.tile(mc.reshape(ROWS, 4, 1, 32), (1, 1, 4, 1))  # [R,4j,4g,32]
        mjf = mj.reshape(ROWS, 4, 128)                       # [R, j, 128]
        for i in range(ROWS):
            for c in range(2):
                w = 256 * i + 128 * c
                mrow_full[cc, 32, w:w + 128] = mjf[i, 2 * c]
                mrow_full[cc, 33, w:w + 128] = 1.0
                mrow_full[cc, 34, w:w + 128] = mjf[i, 2 * c + 1]
                mrow_full[cc, 35, w:w + 128] = 1.0

    if "nc" not in _CACHE:
        _CACHE["nc"] = _build()
    nc = _CACHE["nc"]

    in_maps = []
    for c in range(NCORES):
        in_maps.append({
            "m2": m2,
            "offs": np.ascontiguousarray(offs[c]),
            "mrow": np.ascontiguousarray(mrow_full[c]),
            "rhs": rhs,
        })

    trace = os.environ.get("KERNEL_TRACE", "0") == "1" and _install_profhook()
    res = bass_utils.run_bass_kernel_spmd(
        nc, in_maps, core_ids=list(range(NCORES)), trace=bool(trace))
    if trace:
        _CACHE["exec_time_ns"] = res.exec_time_ns
        _CACHE["trace_path"] = (res.instructions_and_trace or (None, None))[1]
        _CACHE["tmpdir"] = res.profile_json

    # undo the device layout: [i, j, g, b, s, lp, e] -> [i, (j b), (g s lp), e]
    out = np.concatenate(
        [res.results[c]["out"].reshape(ROWS, 4, 4, 32, 4, 8, E)
         .transpose(0, 1, 3, 2, 4, 5, 6).reshape(ROWS, M, L, E)
         for c in range(NCORES)],
        axis=0).astype(np.float32)
    return out
